# revision 3
# baseline (speedup 1.0000x reference)
"""Trainium2 Bass kernel for nn_AdditiveAttention (attention-MIL pooling).

Full inputs in, full outputs out. Internally: data-parallel over the B=16
slide dim across 8 NeuronCores (2 slides/core); MLP weights replicated.

Per core, per slide (N=4096 patches, L=1024, D=256):
  scores  s = W3 . lrelu(W2 . lrelu(BN(W1 . x)))     (feature-major on chip)
  weights w = exp(s) * mask ; denom = sum(w)
  pool    M = (sum_n w_n x_n) / denom
  logits  = C3 . relu(C2 . relu(C1 . M))  -> sigmoid softmax + argmax

On-chip layout: x is cast to bf16 on host. The scores path consumes
x^T tiles (partition = L-chunk) loaded straight from DRAM with the
HWDGE xbar DMA-transpose; the pooling path consumes natural-layout
tiles (partition = patch). All big matmuls run in bf16 on the PE;
the tiny classifier tail runs in fp32.
"""

import numpy as np
import ml_dtypes

B, N, L, D = 16, 4096, 1024, 256
NCORES = 8
BLOC = B // NCORES          # slides per core
PG = 512                    # patches per pipeline group
NGRP = N // PG              # groups per slide
EPS = 1e-5

_BF16 = ml_dtypes.bfloat16

_CACHE = {}


def _build_nc():
    import concourse.bacc as bacc
    import concourse.tile as tile
    import concourse.mybir as mybir

    dt = mybir.dt
    AF = mybir.ActivationFunctionType
    OP = mybir.AluOpType
    AX = mybir.AxisListType

    nc = bacc.Bacc("TRN2", target_bir_lowering=False, debug=False,
                   num_devices=NCORES)

    # ---- per-core inputs -------------------------------------------------
    xn = nc.dram_tensor("xn", [BLOC, N, L], dt.bfloat16, kind="ExternalInput")
    maskf = nc.dram_tensor("maskf", [BLOC, 1, N], dt.float32, kind="ExternalInput")
    w1 = nc.dram_tensor("w1", [L, D], dt.bfloat16, kind="ExternalInput")
    w2 = nc.dram_tensor("w2", [D, D], dt.bfloat16, kind="ExternalInput")
    w3p = nc.dram_tensor("w3p", [128, 2], dt.bfloat16, kind="ExternalInput")
    abp = nc.dram_tensor("abp", [128, 2, 2], dt.float32, kind="ExternalInput")
    b2p = nc.dram_tensor("b2p", [128, 2], dt.float32, kind="ExternalInput")
    b3p = nc.dram_tensor("b3p", [1, 1], dt.float32, kind="ExternalInput")
    c1 = nc.dram_tensor("c1", [L, D], dt.float32, kind="ExternalInput")
    c2 = nc.dram_tensor("c2", [D, D], dt.float32, kind="ExternalInput")
    c3p = nc.dram_tensor("c3p", [128, 2, 2], dt.float32, kind="ExternalInput")
    c1r = nc.dram_tensor("c1r", [BLOC, D], dt.float32, kind="ExternalInput")
    c2r = nc.dram_tensor("c2r", [BLOC, D], dt.float32, kind="ExternalInput")
    c3r = nc.dram_tensor("c3r", [BLOC, 2], dt.float32, kind="ExternalInput")
    idb = nc.dram_tensor("idb", [1, 1], dt.bfloat16, kind="ExternalInput")
    idf = nc.dram_tensor("idf", [1, 1], dt.float32, kind="ExternalInput")
    id2 = nc.dram_tensor("id2", [2, 2], dt.float32, kind="ExternalInput")

    yprob = nc.dram_tensor("yprob", [BLOC, 2], dt.float32, kind="ExternalOutput")
    yhat = nc.dram_tensor("yhat", [BLOC, 1], dt.int32, kind="ExternalOutput")

    with tile.TileContext(nc) as tc:
        with (
            tc.tile_pool(name="consts", bufs=1) as cst,
            tc.tile_pool(name="xnat", bufs=3) as xnp,
            tc.tile_pool(name="xtr", bufs=3) as xtp,
            tc.tile_pool(name="acts", bufs=3) as actp,
            tc.tile_pool(name="small", bufs=3) as smp,
            tc.tile_pool(name="tail", bufs=2) as tlp,
            tc.tile_pool(name="psmm", bufs=2, space="PSUM") as psmm,
            tc.tile_pool(name="pss", bufs=1, space="PSUM") as pss,
            tc.tile_pool(name="pswt", bufs=1, space="PSUM") as pswt,
            tc.tile_pool(name="psM", bufs=1, space="PSUM") as psM,
        ):
            # ---- load replicated constants -------------------------------
            w1t = cst.tile([128, 8, D], dt.bfloat16)
            nc.gpsimd.dma_start(w1t[:], w1.ap().rearrange("(c p) d -> p c d", p=128))
            w2t = cst.tile([128, 2, D], dt.bfloat16)
            nc.gpsimd.dma_start(w2t[:], w2.ap().rearrange("(c p) d -> p c d", p=128))
            w3t = cst.tile([128, 2], dt.bfloat16)
            nc.gpsimd.dma_start(w3t[:], w3p.ap())
            abt = cst.tile([128, 2, 2], dt.float32)
            nc.gpsimd.dma_start(abt[:], abp.ap())
            b2t = cst.tile([128, 2], dt.float32)
            nc.gpsimd.dma_start(b2t[:], b2p.ap())
            b3t = cst.tile([1, 1], dt.float32)
            nc.gpsimd.dma_start(b3t[:], b3p.ap())
            c1t = cst.tile([128, 8, D], dt.float32)
            nc.gpsimd.dma_start(c1t[:], c1.ap().rearrange("(c p) d -> p c d", p=128))
            c2t = cst.tile([128, 2, D], dt.float32)
            nc.gpsimd.dma_start(c2t[:], c2.ap().rearrange("(c p) d -> p c d", p=128))
            c3t = cst.tile([128, 2, 2], dt.float32)
            nc.gpsimd.dma_start(c3t[:], c3p.ap())
            c1rt = cst.tile([BLOC, D], dt.float32)
            nc.gpsimd.dma_start(c1rt[:], c1r.ap())
            c2rt = cst.tile([BLOC, D], dt.float32)
            nc.gpsimd.dma_start(c2rt[:], c2r.ap())
            c3rt = cst.tile([BLOC, 2], dt.float32)
            nc.gpsimd.dma_start(c3rt[:], c3r.ap())
            idbt = cst.tile([1, 1], dt.bfloat16)
            nc.gpsimd.dma_start(idbt[:], idb.ap())
            idft = cst.tile([1, 1], dt.float32)
            nc.gpsimd.dma_start(idft[:], idf.ap())
            id2t = cst.tile([2, 2], dt.float32)
            nc.gpsimd.dma_start(id2t[:], id2.ap())

            # normalized M^T columns for both slides: [128, chunk, slide]
            mtall = cst.tile([128, 8, BLOC], dt.float32)

            for s in range(BLOC):
                maskrow = smp.tile([1, N], dt.float32, tag="maskrow")
                nc.gpsimd.dma_start(maskrow[:], maskf[s])

                mps = psM.tile([1, L], dt.float32, tag="Mpool")
                dent = smp.tile([1, NGRP], dt.float32, tag="dent")

                for g in range(NGRP):
                    # ---- loads ------------------------------------------
                    xnt = xnp.tile([128, 4, L], dt.bfloat16, tag="xn")
                    nc.gpsimd.dma_start(
                        xnt[:],
                        xn[s].rearrange("(G p) l -> p G l", p=128)[:, 4 * g:4 * g + 4, :],
                    )
                    xtt = xtp.tile([128, 8, PG], dt.bfloat16, tag="xt")
                    for c in range(8):
                        nc.sync.dma_start_transpose(
                            xtt[:, c, :],
                            xn[s, PG * g:PG * (g + 1), 128 * c:128 * (c + 1)],
                        )

                    # ---- layer 1: hT = x @ W1 (feature-major) -----------
                    hps = psmm.tile([128, 2, PG], dt.float32, tag="mm")
                    for dh in range(2):
                        for c in range(8):
                            nc.tensor.matmul(
                                hps[:, dh, :],
                                lhsT=w1t[:, c, 128 * dh:128 * (dh + 1)],
                                rhs=xtt[:, c, :],
                                start=(c == 0), stop=(c == 7),
                            )
                    hs = actp.tile([128, 2, PG], dt.bfloat16, tag="hs")
                    for dh in range(2):
                        nc.scalar.activation(
                            hs[:, dh, :], hps[:, dh, :], AF.Prelu,
                            bias=abt[:, dh, 1:2], scale=abt[:, dh, 0:1], alpha=0.2,
                        )

                    # ---- layer 2 ----------------------------------------
                    gps = psmm.tile([128, 2, PG], dt.float32, tag="mm")
                    for eh in range(2):
                        for c in range(2):
                            nc.tensor.matmul(
                                gps[:, eh, :],
                                lhsT=w2t[:, c, 128 * eh:128 * (eh + 1)],
                                rhs=hs[:, c, :],
                                start=(c == 0), stop=(c == 1),
                            )
                    gs = actp.tile([128, 2, PG], dt.bfloat16, tag="gs")
                    for eh in range(2):
                        nc.scalar.activation(
                            gs[:, eh, :], gps[:, eh, :], AF.Prelu,
                            bias=b2t[:, eh:eh+1], scale=1.0, alpha=0.2,
                        )

                    # ---- layer 3: scores row [1, PG] --------------------
                    sps = pss.tile([1, PG], dt.float32, tag="srow")
                    for c in range(2):
                        nc.tensor.matmul(
                            sps[:],
                            lhsT=w3t[:, c:c + 1],
                            rhs=gs[:, c, :],
                            start=(c == 0), stop=(c == 1),
                        )

                    # ---- w = exp(s + b3) * mask;  denom partial ---------
                    wrow = smp.tile([1, PG], dt.float32, tag="wrow")
                    nc.scalar.activation(wrow[:], sps[:], AF.Exp,
                                         bias=b3t[0:1, 0:1], scale=1.0)
                    wrowf = smp.tile([1, PG], dt.float32, tag="wrowf")
                    nc.vector.scalar_tensor_tensor(
                        out=wrowf[:], in0=wrow[:], scalar=1.0,
                        in1=maskrow[0:1, PG * g:PG * (g + 1)],
                        op0=OP.mult, op1=OP.mult,
                        accum_out=dent[0:1, g:g + 1],
                    )

                    # ---- w row -> columns (PE transpose) ----------------
                    wtp = pswt.tile([128, 4], dt.float32, tag="wT")
                    for q in range(4):
                        nc.tensor.transpose(
                            wtp[:, q:q + 1],
                            wrowf[0:1, 128 * q:128 * (q + 1)],
                            idft[:],
                        )
                    wsb = smp.tile([128, 4], dt.bfloat16, tag="wsb")
                    nc.vector.tensor_copy(wsb[:], wtp[:])

                    # ---- pooling: M += w^T . x --------------------------
                    for q in range(4):
                        for h in range(2):
                            nc.tensor.matmul(
                                mps[0:1, PG * h:PG * (h + 1)],
                                lhsT=wsb[:, q:q + 1],
                                rhs=xnt[:, q, PG * h:PG * (h + 1)],
                                start=(g == 0 and q == 0),
                                stop=(g == NGRP - 1 and q == 3),
                                skip_group_check=True,
                            )

                # ---- slide tail: normalize M, transpose to columns ------
                den = tlp.tile([1, 1], dt.float32, tag="den")
                nc.vector.reduce_sum(den[:], dent[:], axis=AX.X)
                rden = tlp.tile([1, 1], dt.float32, tag="rden")
                nc.vector.reciprocal(rden[:], den[:])
                mn = tlp.tile([1, L], dt.float32, tag="mn")
                nc.scalar.activation(mn[:], mps[:], AF.Copy, bias=0.0,
                                     scale=rden[0:1, 0:1])
                mtp = psmm.tile([128, 8, 1], dt.float32, tag="mm")
                for c in range(8):
                    nc.tensor.transpose(
                        mtp[:, c:c + 1], mn[0:1, 128 * c:128 * (c + 1)], idft[:],
                    )
                nc.vector.tensor_copy(mtall[:, :, s:s+1], mtp[:])

            # ---- classifier (both slides at once, fp32) -----------------
            z1p = psmm.tile([BLOC, D], dt.float32, tag="mm")
            for c in range(8):
                nc.tensor.matmul(z1p[:], lhsT=mtall[:, c, :], rhs=c1t[:, c, :],
                                 start=(c == 0), stop=(c == 7))
            z1a = tlp.tile([BLOC, D], dt.float32, tag="z1a")
            nc.vector.tensor_tensor(out=z1a[:], in0=z1p[:], in1=c1rt[:],
                                    op=OP.add)
            z1s = tlp.tile([BLOC, D], dt.float32, tag="z1s")
            nc.vector.tensor_scalar_max(z1s[:], z1a[:], 0.0)
            z1tp = psmm.tile([128, 2, BLOC], dt.float32, tag="mm")
            for c in range(2):
                nc.tensor.transpose(z1tp[:, c, :], z1s[:, 128 * c:128 * (c + 1)],
                                    id2t[:])
            z1tsb = tlp.tile([128, 2, BLOC], dt.float32, tag="z1t")
            nc.vector.tensor_copy(z1tsb[:], z1tp[:])

            z2p = pss.tile([BLOC, D], dt.float32, tag="srow")
            for c in range(2):
                nc.tensor.matmul(z2p[:], lhsT=z1tsb[:, c, :], rhs=c2t[:, c, :],
                                 start=(c == 0), stop=(c == 1))
            z2a = tlp.tile([BLOC, D], dt.float32, tag="z2a")
            nc.vector.tensor_tensor(out=z2a[:], in0=z2p[:], in1=c2rt[:],
                                    op=OP.add)
            z2s = tlp.tile([BLOC, D], dt.float32, tag="z2s")
            nc.vector.tensor_scalar_max(z2s[:], z2a[:], 0.0)
            z2tp = psmm.tile([128, 2, BLOC], dt.float32, tag="mm")
            for c in range(2):
                nc.tensor.transpose(z2tp[:, c, :], z2s[:, 128 * c:128 * (c + 1)],
                                    id2t[:])
            z2tsb = tlp.tile([128, 2, BLOC], dt.float32, tag="z2t")
            nc.vector.tensor_copy(z2tsb[:], z2tp[:])

            lgp = pswt.tile([BLOC, 2], dt.float32, tag="wT")
            for c in range(2):
                nc.tensor.matmul(lgp[:], lhsT=z2tsb[:, c, :], rhs=c3t[:, c, :],
                                 start=(c == 0), stop=(c == 1))
            lg = tlp.tile([BLOC, 2], dt.float32, tag="lg")
            nc.vector.tensor_tensor(out=lg[:], in0=lgp[:], in1=c3rt[:], op=OP.add)

            diff = tlp.tile([BLOC, 1], dt.float32, tag="diff")
            nc.vector.tensor_tensor(out=diff[:], in0=lg[:, 1:2], in1=lg[:, 0:1],
                                    op=OP.subtract)
            ex = tlp.tile([BLOC, 1], dt.float32, tag="ex")
            nc.scalar.activation(ex[:], diff[:], AF.Exp, bias=0.0, scale=1.0)
            t1 = tlp.tile([BLOC, 1], dt.float32, tag="t1")
            nc.vector.tensor_scalar_add(t1[:], ex[:], 1.0)
            rr = tlp.tile([BLOC, 1], dt.float32, tag="rr")
            nc.vector.reciprocal(rr[:], t1[:])

            ypt = tlp.tile([BLOC, 2], dt.float32, tag="ypt")
            nc.vector.tensor_copy(ypt[:, 0:1], rr[:])          # p0 = 1/(1+e)
            nc.vector.tensor_tensor(out=ypt[:, 1:2], in0=ex[:], in1=rr[:],
                                    op=OP.mult)                # p1 = e/(1+e)
            yhf = tlp.tile([BLOC, 1], dt.float32, tag="yhf")
            nc.vector.tensor_scalar(out=yhf[:], in0=diff[:], scalar1=0.0,
                                    scalar2=None, op0=OP.is_gt)
            yht = tlp.tile([BLOC, 1], dt.int32, tag="yht")
            nc.vector.tensor_copy(yht[:], yhf[:])

            nc.sync.dma_start(yprob.ap(), ypt[:])
            nc.sync.dma_start(yhat.ap(), yht[:])

    nc.compile()
    return nc


def _prep(inputs):
    f32 = np.float32
    x = np.ascontiguousarray(np.asarray(inputs["x"], f32))
    mask = np.asarray(inputs["mask"])
    W1 = np.asarray(inputs["W1"], f32)
    b1 = np.asarray(inputs["b1"], f32)
    gam = np.asarray(inputs["bn_gamma"], f32)
    bet = np.asarray(inputs["bn_beta"], f32)
    mu = np.asarray(inputs["bn_mean"], f32)
    var = np.asarray(inputs["bn_var"], f32)
    W2 = np.asarray(inputs["W2"], f32)
    b2 = np.asarray(inputs["b2"], f32)
    W3 = np.asarray(inputs["W3"], f32)
    b3 = np.asarray(inputs["b3"], f32)
    C1 = np.asarray(inputs["C1"], f32)
    c1 = np.asarray(inputs["c1"], f32)
    C2 = np.asarray(inputs["C2"], f32)
    c2 = np.asarray(inputs["c2"], f32)
    C3 = np.asarray(inputs["C3"], f32)
    c3 = np.asarray(inputs["c3"], f32)

    A = gam / np.sqrt(var + EPS)                    # BN scale
    Bv = (b1 - mu) * A + bet                        # BN bias (b1 folded)

    xb = x.astype(_BF16)
    shared = {
        "w1": W1.astype(_BF16),
        "w2": W2.astype(_BF16),
        "w3p": np.ascontiguousarray(W3.reshape(2, 128).T).astype(_BF16),
        "abp": np.ascontiguousarray(
            np.stack([A.reshape(2, 128).T, Bv.reshape(2, 128).T], axis=2)),
        "b2p": np.ascontiguousarray(b2.reshape(2, 128).T),
        "b3p": np.full((1, 1), b3[0], f32),
        "c1": C1,
        "c2": C2,
        "c3p": np.ascontiguousarray(C3.reshape(2, 128, 2).transpose(1, 0, 2)),
        "c1r": np.broadcast_to(c1, (BLOC, D)).copy(),
        "c2r": np.broadcast_to(c2, (BLOC, D)).copy(),
        "c3r": np.broadcast_to(c3, (BLOC, 2)).copy(),
        "idb": np.ones((1, 1), _BF16),
        "idf": np.ones((1, 1), f32),
        "id2": np.eye(2, dtype=f32),
    }
    in_maps = []
    for k in range(NCORES):
        sl = slice(BLOC * k, BLOC * (k + 1))
        m = dict(shared)
        m["xn"] = np.ascontiguousarray(xb[sl])
        m["maskf"] = np.ascontiguousarray(
            mask[sl].astype(f32).reshape(BLOC, 1, N))
        in_maps.append(m)
    return in_maps


def kernel(**inputs):
    from concourse.bass_utils import run_bass_kernel_spmd

    if "nc" not in _CACHE:
        _CACHE["nc"] = _build_nc()
    nc = _CACHE["nc"]

    in_maps = _prep(inputs)
    res = run_bass_kernel_spmd(nc, in_maps, core_ids=list(range(NCORES)))
    yprob = np.concatenate([r["yprob"] for r in res.results], axis=0)
    yhat = np.concatenate([r["yhat"][:, 0] for r in res.results], axis=0)
    return yprob.astype(np.float32), yhat.astype(np.int32)


# revision 5
# speedup vs baseline: 1.4180x; 1.4180x over previous
"""Trainium2 Bass kernel for nn_AdditiveAttention (attention-MIL pooling).

Full inputs in, full outputs out. Internally: data-parallel over the B=16
slide dim across 8 NeuronCores (2 slides/core); MLP weights replicated.

Per core, per slide (N=4096 patches, L=1024, D=256):
  scores  s = W3 . lrelu(W2 . lrelu(BN(W1 . x)))     (feature-major on chip)
  weights w = exp(s) * mask ; denom = sum(w)
  pool    M = (sum_n w_n x_n) / denom
  logits  = C3 . relu(C2 . relu(C1 . M))  -> sigmoid softmax + argmax

On-chip layout: x is cast to bf16 on host. The scores path consumes
x^T tiles (partition = L-chunk) loaded straight from DRAM with the
HWDGE xbar DMA-transpose; the pooling path consumes natural-layout
tiles (partition = patch). All big matmuls run in bf16 on the PE;
the tiny classifier tail runs in fp32.
"""

import numpy as np
import ml_dtypes

B, N, L, D = 16, 4096, 1024, 256
NCORES = 8
BLOC = B // NCORES          # slides per core
PG = 512                    # patches per pipeline group
NGRP = N // PG              # groups per slide
EPS = 1e-5

_BF16 = ml_dtypes.bfloat16

_CACHE = {}


def _build_nc():
    import concourse.bacc as bacc
    import concourse.tile as tile
    import concourse.mybir as mybir

    dt = mybir.dt
    AF = mybir.ActivationFunctionType
    OP = mybir.AluOpType
    AX = mybir.AxisListType

    nc = bacc.Bacc("TRN2", target_bir_lowering=False, debug=False,
                   num_devices=NCORES)

    # ---- per-core inputs -------------------------------------------------
    xn = nc.dram_tensor("xn", [BLOC, N, L], dt.bfloat16, kind="ExternalInput")
    maskf = nc.dram_tensor("maskf", [BLOC, 1, N], dt.bfloat16, kind="ExternalInput")
    w1 = nc.dram_tensor("w1", [L, D], dt.bfloat16, kind="ExternalInput")
    w2 = nc.dram_tensor("w2", [D, D], dt.bfloat16, kind="ExternalInput")
    w3p = nc.dram_tensor("w3p", [128, 2], dt.bfloat16, kind="ExternalInput")
    abp = nc.dram_tensor("abp", [128, 2, 2], dt.float32, kind="ExternalInput")
    b2p = nc.dram_tensor("b2p", [128, 2], dt.float32, kind="ExternalInput")
    b3p = nc.dram_tensor("b3p", [1, 1], dt.float32, kind="ExternalInput")
    c1 = nc.dram_tensor("c1", [L, D], dt.float32, kind="ExternalInput")
    c2 = nc.dram_tensor("c2", [D, D], dt.float32, kind="ExternalInput")
    c3p = nc.dram_tensor("c3p", [128, 2, 2], dt.float32, kind="ExternalInput")
    c1r = nc.dram_tensor("c1r", [BLOC, D], dt.float32, kind="ExternalInput")
    c2r = nc.dram_tensor("c2r", [BLOC, D], dt.float32, kind="ExternalInput")
    c3r = nc.dram_tensor("c3r", [BLOC, 2], dt.float32, kind="ExternalInput")
    idb = nc.dram_tensor("idb", [1, 1], dt.bfloat16, kind="ExternalInput")
    idf = nc.dram_tensor("idf", [1, 1], dt.float32, kind="ExternalInput")
    id2 = nc.dram_tensor("id2", [2, 2], dt.float32, kind="ExternalInput")

    yprob = nc.dram_tensor("yprob", [BLOC, 2], dt.float32, kind="ExternalOutput")
    yhat = nc.dram_tensor("yhat", [BLOC, 1], dt.int32, kind="ExternalOutput")

    with tile.TileContext(nc) as tc:
        with (
            tc.tile_pool(name="consts", bufs=1) as cst,
            tc.tile_pool(name="xnat", bufs=3) as xnp,
            tc.tile_pool(name="xtr", bufs=2) as xtp,
            tc.tile_pool(name="mrow", bufs=2) as mrp,
            tc.tile_pool(name="acts", bufs=3) as actp,
            tc.tile_pool(name="small", bufs=3) as smp,
            tc.tile_pool(name="tail", bufs=2) as tlp,
            tc.tile_pool(name="psmm", bufs=2, space="PSUM") as psmm,
            tc.tile_pool(name="pss", bufs=1, space="PSUM") as pss,
            tc.tile_pool(name="pswt", bufs=1, space="PSUM") as pswt,
            tc.tile_pool(name="psM", bufs=1, space="PSUM") as psM,
        ):
            # ---- load replicated constants -------------------------------
            w1t = cst.tile([128, 8, D], dt.bfloat16)
            nc.gpsimd.dma_start(w1t[:], w1.ap().rearrange("(c p) d -> p c d", p=128))
            w2t = cst.tile([128, 2, D], dt.bfloat16)
            nc.gpsimd.dma_start(w2t[:], w2.ap().rearrange("(c p) d -> p c d", p=128))
            w3t = cst.tile([128, 2], dt.bfloat16)
            nc.gpsimd.dma_start(w3t[:], w3p.ap())
            abt = cst.tile([128, 2, 2], dt.float32)
            nc.gpsimd.dma_start(abt[:], abp.ap())
            b2t = cst.tile([128, 2], dt.float32)
            nc.gpsimd.dma_start(b2t[:], b2p.ap())
            b3t = cst.tile([1, 1], dt.float32)
            nc.gpsimd.dma_start(b3t[:], b3p.ap())
            c1t = cst.tile([128, 8, D], dt.float32)
            nc.gpsimd.dma_start(c1t[:], c1.ap().rearrange("(c p) d -> p c d", p=128))
            c2t = cst.tile([128, 2, D], dt.float32)
            nc.gpsimd.dma_start(c2t[:], c2.ap().rearrange("(c p) d -> p c d", p=128))
            c3t = cst.tile([128, 2, 2], dt.float32)
            nc.gpsimd.dma_start(c3t[:], c3p.ap())
            c1rt = cst.tile([BLOC, D], dt.float32)
            nc.gpsimd.dma_start(c1rt[:], c1r.ap())
            c2rt = cst.tile([BLOC, D], dt.float32)
            nc.gpsimd.dma_start(c2rt[:], c2r.ap())
            c3rt = cst.tile([BLOC, 2], dt.float32)
            nc.gpsimd.dma_start(c3rt[:], c3r.ap())
            idbt = cst.tile([1, 1], dt.bfloat16)
            nc.gpsimd.dma_start(idbt[:], idb.ap())
            idft = cst.tile([1, 1], dt.float32)
            nc.gpsimd.dma_start(idft[:], idf.ap())
            id2t = cst.tile([2, 2], dt.float32)
            nc.gpsimd.dma_start(id2t[:], id2.ap())

            # normalized M^T columns for both slides: [128, chunk, slide]
            mtall = cst.tile([128, 8, BLOC], dt.float32)

            for s in range(BLOC):
                maskrow = mrp.tile([1, N], dt.bfloat16, tag="maskrow")
                nc.gpsimd.dma_start(maskrow[:], maskf[s])

                mps = psM.tile([1, L], dt.float32, tag="Mpool")
                dent = smp.tile([1, NGRP], dt.float32, tag="dent")

                xthalf = [None, None]
                for g in range(NGRP):
                    # ---- loads ------------------------------------------
                    xnt = xnp.tile([128, 4, L], dt.bfloat16, tag="xn")
                    nc.gpsimd.dma_start(
                        xnt[:],
                        xn[s].rearrange("(G p) l -> p G l", p=128)[:, 4 * g:4 * g + 4, :],
                    )
                    HPG = PG * NGRP // 2          # patches per half-slide
                    h_idx = (PG * g) // HPG
                    if xthalf[h_idx] is None or (PG * g) % HPG == 0:
                        xth = xtp.tile([128, 8, HPG], dt.bfloat16, tag="xt")
                        for c in range(8):
                            nc.sync.dma_start_transpose(
                                xth[:, c, :],
                                xn[s, HPG * h_idx:HPG * (h_idx + 1),
                                   128 * c:128 * (c + 1)],
                            )
                        xthalf[h_idx] = xth
                    xtt = xthalf[h_idx]
                    goff = (PG * g) % HPG         # patch offset within half tile

                    # ---- layer 1: hT = x @ W1 (feature-major) -----------
                    hps = psmm.tile([128, 2, PG], dt.float32, tag="mm")
                    for dh in range(2):
                        for c in range(8):
                            nc.tensor.matmul(
                                hps[:, dh, :],
                                lhsT=w1t[:, c, 128 * dh:128 * (dh + 1)],
                                rhs=xtt[:, c, goff:goff + PG],
                                start=(c == 0), stop=(c == 7),
                            )
                    hs = actp.tile([128, 2, PG], dt.bfloat16, tag="hs")
                    for dh in range(2):
                        nc.scalar.activation(
                            hs[:, dh, :], hps[:, dh, :], AF.Prelu,
                            bias=abt[:, dh, 1:2], scale=abt[:, dh, 0:1], alpha=0.2,
                        )

                    # ---- layer 2 ----------------------------------------
                    gps = psmm.tile([128, 2, PG], dt.float32, tag="mm")
                    for eh in range(2):
                        for c in range(2):
                            nc.tensor.matmul(
                                gps[:, eh, :],
                                lhsT=w2t[:, c, 128 * eh:128 * (eh + 1)],
                                rhs=hs[:, c, :],
                                start=(c == 0), stop=(c == 1),
                            )
                    gs = actp.tile([128, 2, PG], dt.bfloat16, tag="gs")
                    for eh in range(2):
                        nc.scalar.activation(
                            gs[:, eh, :], gps[:, eh, :], AF.Prelu,
                            bias=b2t[:, eh:eh+1], scale=1.0, alpha=0.2,
                        )

                    # ---- layer 3: scores row [1, PG] --------------------
                    sps = pss.tile([1, PG], dt.float32, tag="srow")
                    for c in range(2):
                        nc.tensor.matmul(
                            sps[:],
                            lhsT=w3t[:, c:c + 1],
                            rhs=gs[:, c, :],
                            start=(c == 0), stop=(c == 1),
                        )

                    # ---- w = exp(s + b3) * mask;  denom partial ---------
                    wrow = smp.tile([1, PG], dt.float32, tag="wrow")
                    nc.scalar.activation(wrow[:], sps[:], AF.Exp,
                                         bias=b3t[0:1, 0:1], scale=1.0)
                    wrowf = smp.tile([1, PG], dt.float32, tag="wrowf")
                    nc.vector.scalar_tensor_tensor(
                        out=wrowf[:], in0=wrow[:], scalar=1.0,
                        in1=maskrow[0:1, PG * g:PG * (g + 1)],
                        op0=OP.mult, op1=OP.mult,
                        accum_out=dent[0:1, g:g + 1],
                    )

                    # ---- w row -> columns (PE transpose) ----------------
                    wtp = pswt.tile([128, 4], dt.float32, tag="wT")
                    for q in range(4):
                        nc.tensor.transpose(
                            wtp[:, q:q + 1],
                            wrowf[0:1, 128 * q:128 * (q + 1)],
                            idft[:],
                        )
                    wsb = smp.tile([128, 4], dt.bfloat16, tag="wsb")
                    nc.vector.tensor_copy(wsb[:], wtp[:])

                    # ---- pooling: M += w^T . x --------------------------
                    for q in range(4):
                        for h in range(2):
                            nc.tensor.matmul(
                                mps[0:1, PG * h:PG * (h + 1)],
                                lhsT=wsb[:, q:q + 1],
                                rhs=xnt[:, q, PG * h:PG * (h + 1)],
                                start=(g == 0 and q == 0),
                                stop=(g == NGRP - 1 and q == 3),
                                skip_group_check=True,
                            )

                # ---- slide tail: normalize M, transpose to columns ------
                den = tlp.tile([1, 1], dt.float32, tag="den")
                nc.vector.reduce_sum(den[:], dent[:], axis=AX.X)
                rden = tlp.tile([1, 1], dt.float32, tag="rden")
                nc.vector.reciprocal(rden[:], den[:])
                mn = tlp.tile([1, L], dt.float32, tag="mn")
                nc.scalar.activation(mn[:], mps[:], AF.Copy, bias=0.0,
                                     scale=rden[0:1, 0:1])
                mtp = psmm.tile([128, 8, 1], dt.float32, tag="mm")
                for c in range(8):
                    nc.tensor.transpose(
                        mtp[:, c:c + 1], mn[0:1, 128 * c:128 * (c + 1)], idft[:],
                    )
                nc.vector.tensor_copy(mtall[:, :, s:s+1], mtp[:])

            # ---- classifier (both slides at once, fp32) -----------------
            z1p = psmm.tile([BLOC, D], dt.float32, tag="mm")
            for c in range(8):
                nc.tensor.matmul(z1p[:], lhsT=mtall[:, c, :], rhs=c1t[:, c, :],
                                 start=(c == 0), stop=(c == 7))
            z1a = tlp.tile([BLOC, D], dt.float32, tag="z1a")
            nc.vector.tensor_tensor(out=z1a[:], in0=z1p[:], in1=c1rt[:],
                                    op=OP.add)
            z1s = tlp.tile([BLOC, D], dt.float32, tag="z1s")
            nc.vector.tensor_scalar_max(z1s[:], z1a[:], 0.0)
            z1tp = psmm.tile([128, 2, BLOC], dt.float32, tag="mm")
            for c in range(2):
                nc.tensor.transpose(z1tp[:, c, :], z1s[:, 128 * c:128 * (c + 1)],
                                    id2t[:])
            z1tsb = tlp.tile([128, 2, BLOC], dt.float32, tag="z1t")
            nc.vector.tensor_copy(z1tsb[:], z1tp[:])

            z2p = pss.tile([BLOC, D], dt.float32, tag="srow")
            for c in range(2):
                nc.tensor.matmul(z2p[:], lhsT=z1tsb[:, c, :], rhs=c2t[:, c, :],
                                 start=(c == 0), stop=(c == 1))
            z2a = tlp.tile([BLOC, D], dt.float32, tag="z2a")
            nc.vector.tensor_tensor(out=z2a[:], in0=z2p[:], in1=c2rt[:],
                                    op=OP.add)
            z2s = tlp.tile([BLOC, D], dt.float32, tag="z2s")
            nc.vector.tensor_scalar_max(z2s[:], z2a[:], 0.0)
            z2tp = psmm.tile([128, 2, BLOC], dt.float32, tag="mm")
            for c in range(2):
                nc.tensor.transpose(z2tp[:, c, :], z2s[:, 128 * c:128 * (c + 1)],
                                    id2t[:])
            z2tsb = tlp.tile([128, 2, BLOC], dt.float32, tag="z2t")
            nc.vector.tensor_copy(z2tsb[:], z2tp[:])

            lgp = pswt.tile([BLOC, 2], dt.float32, tag="wT")
            for c in range(2):
                nc.tensor.matmul(lgp[:], lhsT=z2tsb[:, c, :], rhs=c3t[:, c, :],
                                 start=(c == 0), stop=(c == 1))
            lg = tlp.tile([BLOC, 2], dt.float32, tag="lg")
            nc.vector.tensor_tensor(out=lg[:], in0=lgp[:], in1=c3rt[:], op=OP.add)

            diff = tlp.tile([BLOC, 1], dt.float32, tag="diff")
            nc.vector.tensor_tensor(out=diff[:], in0=lg[:, 1:2], in1=lg[:, 0:1],
                                    op=OP.subtract)
            ex = tlp.tile([BLOC, 1], dt.float32, tag="ex")
            nc.scalar.activation(ex[:], diff[:], AF.Exp, bias=0.0, scale=1.0)
            t1 = tlp.tile([BLOC, 1], dt.float32, tag="t1")
            nc.vector.tensor_scalar_add(t1[:], ex[:], 1.0)
            rr = tlp.tile([BLOC, 1], dt.float32, tag="rr")
            nc.vector.reciprocal(rr[:], t1[:])

            ypt = tlp.tile([BLOC, 2], dt.float32, tag="ypt")
            nc.vector.tensor_copy(ypt[:, 0:1], rr[:])          # p0 = 1/(1+e)
            nc.vector.tensor_tensor(out=ypt[:, 1:2], in0=ex[:], in1=rr[:],
                                    op=OP.mult)                # p1 = e/(1+e)
            yhf = tlp.tile([BLOC, 1], dt.float32, tag="yhf")
            nc.vector.tensor_scalar(out=yhf[:], in0=diff[:], scalar1=0.0,
                                    scalar2=None, op0=OP.is_gt)
            yht = tlp.tile([BLOC, 1], dt.int32, tag="yht")
            nc.vector.tensor_copy(yht[:], yhf[:])

            nc.sync.dma_start(yprob.ap(), ypt[:])
            nc.sync.dma_start(yhat.ap(), yht[:])

    nc.compile()
    return nc


def _prep(inputs):
    f32 = np.float32
    x = np.ascontiguousarray(np.asarray(inputs["x"], f32))
    mask = np.asarray(inputs["mask"])
    W1 = np.asarray(inputs["W1"], f32)
    b1 = np.asarray(inputs["b1"], f32)
    gam = np.asarray(inputs["bn_gamma"], f32)
    bet = np.asarray(inputs["bn_beta"], f32)
    mu = np.asarray(inputs["bn_mean"], f32)
    var = np.asarray(inputs["bn_var"], f32)
    W2 = np.asarray(inputs["W2"], f32)
    b2 = np.asarray(inputs["b2"], f32)
    W3 = np.asarray(inputs["W3"], f32)
    b3 = np.asarray(inputs["b3"], f32)
    C1 = np.asarray(inputs["C1"], f32)
    c1 = np.asarray(inputs["c1"], f32)
    C2 = np.asarray(inputs["C2"], f32)
    c2 = np.asarray(inputs["c2"], f32)
    C3 = np.asarray(inputs["C3"], f32)
    c3 = np.asarray(inputs["c3"], f32)

    A = gam / np.sqrt(var + EPS)                    # BN scale
    Bv = (b1 - mu) * A + bet                        # BN bias (b1 folded)

    xb = x.astype(_BF16)
    shared = {
        "w1": W1.astype(_BF16),
        "w2": W2.astype(_BF16),
        "w3p": np.ascontiguousarray(W3.reshape(2, 128).T).astype(_BF16),
        "abp": np.ascontiguousarray(
            np.stack([A.reshape(2, 128).T, Bv.reshape(2, 128).T], axis=2)),
        "b2p": np.ascontiguousarray(b2.reshape(2, 128).T),
        "b3p": np.full((1, 1), b3[0], f32),
        "c1": C1,
        "c2": C2,
        "c3p": np.ascontiguousarray(C3.reshape(2, 128, 2).transpose(1, 0, 2)),
        "c1r": np.broadcast_to(c1, (BLOC, D)).copy(),
        "c2r": np.broadcast_to(c2, (BLOC, D)).copy(),
        "c3r": np.broadcast_to(c3, (BLOC, 2)).copy(),
        "idb": np.ones((1, 1), _BF16),
        "idf": np.ones((1, 1), f32),
        "id2": np.eye(2, dtype=f32),
    }
    in_maps = []
    for k in range(NCORES):
        sl = slice(BLOC * k, BLOC * (k + 1))
        m = dict(shared)
        m["xn"] = np.ascontiguousarray(xb[sl])
        m["maskf"] = np.ascontiguousarray(
            mask[sl].astype(_BF16).reshape(BLOC, 1, N))
        in_maps.append(m)
    return in_maps


def kernel(**inputs):
    from concourse.bass_utils import run_bass_kernel_spmd

    if "nc" not in _CACHE:
        _CACHE["nc"] = _build_nc()
    nc = _CACHE["nc"]

    in_maps = _prep(inputs)
    res = run_bass_kernel_spmd(nc, in_maps, core_ids=list(range(NCORES)))
    yprob = np.concatenate([r["yprob"] for r in res.results], axis=0)
    yhat = np.concatenate([r["yhat"][:, 0] for r in res.results], axis=0)
    return yprob.astype(np.float32), yhat.astype(np.int32)


# revision 6
# speedup vs baseline: 1.8067x; 1.2741x over previous
"""Trainium2 Bass kernel for nn_AdditiveAttention (attention-MIL pooling).

Full inputs in, full outputs out. Internally: data-parallel over the B=16
slide dim across 8 NeuronCores (2 slides/core); MLP weights replicated.

Per core, per slide (N=4096 patches, L=1024, D=256):
  scores  s = W3 . lrelu(W2 . lrelu(BN(W1 . x))) + maskNEG
  weights w = exp(s) ; denom = sum(w)        (masked patches -> w = 0)
  pool    M = (sum_n w_n x_n) / denom
  logits  = C3 . relu(C2 . relu(C1 . M))  -> sigmoid softmax + argmax

On-chip layout: x is cast to bf16 on host and loaded ONLY in transposed
form (partition = L-chunk) via the HWDGE xbar DMA-transpose, so the DMA
stream never mixes transpose and copy modes (they serialize in HW).
The scores path runs feature-major on the PE in bf16.  Pooling runs on
the Vector engine: w is broadcast across partitions with a PE outer
product, then per-chunk multiply with fp32 accum_out gives M^T columns
directly.  The tiny classifier tail runs in fp32 on PE/DVE.
"""

import numpy as np
import ml_dtypes

B, N, L, D = 16, 4096, 1024, 256
NCORES = 8
BLOC = B // NCORES          # slides per core
PG = 512                    # patches per pipeline group
NGRP = N // PG              # groups per slide
HPG = 2048                  # patches per transposed load tile
EPS = 1e-5
MASKNEG = -30000.0

_BF16 = ml_dtypes.bfloat16

_CACHE = {}


def _build_nc():
    import concourse.bacc as bacc
    import concourse.tile as tile
    import concourse.mybir as mybir

    dt = mybir.dt
    AF = mybir.ActivationFunctionType
    OP = mybir.AluOpType
    AX = mybir.AxisListType

    nc = bacc.Bacc("TRN2", target_bir_lowering=False, debug=False,
                   num_devices=NCORES)

    # ---- per-core inputs -------------------------------------------------
    xn = nc.dram_tensor("xn", [BLOC, N, L], dt.bfloat16, kind="ExternalInput")
    maskn = nc.dram_tensor("maskn", [BLOC, 1, N], dt.bfloat16, kind="ExternalInput")
    w1 = nc.dram_tensor("w1", [L, D], dt.bfloat16, kind="ExternalInput")
    w2 = nc.dram_tensor("w2", [D, D], dt.bfloat16, kind="ExternalInput")
    w3p = nc.dram_tensor("w3p", [128, 2], dt.bfloat16, kind="ExternalInput")
    abp = nc.dram_tensor("abp", [128, 2, 2], dt.float32, kind="ExternalInput")
    b2p = nc.dram_tensor("b2p", [128, 2], dt.float32, kind="ExternalInput")
    b3p = nc.dram_tensor("b3p", [1, 1], dt.float32, kind="ExternalInput")
    c1 = nc.dram_tensor("c1", [L, D], dt.float32, kind="ExternalInput")
    c2 = nc.dram_tensor("c2", [D, D], dt.float32, kind="ExternalInput")
    c3p = nc.dram_tensor("c3p", [128, 2, 2], dt.float32, kind="ExternalInput")
    c1r = nc.dram_tensor("c1r", [BLOC, D], dt.float32, kind="ExternalInput")
    c2r = nc.dram_tensor("c2r", [BLOC, D], dt.float32, kind="ExternalInput")
    c3r = nc.dram_tensor("c3r", [BLOC, 2], dt.float32, kind="ExternalInput")
    onesc = nc.dram_tensor("onesc", [1, 128], dt.bfloat16, kind="ExternalInput")
    onescf = nc.dram_tensor("onescf", [1, 128], dt.float32, kind="ExternalInput")
    id2 = nc.dram_tensor("id2", [2, 2], dt.float32, kind="ExternalInput")

    yprob = nc.dram_tensor("yprob", [BLOC, 2], dt.float32, kind="ExternalOutput")
    yhat = nc.dram_tensor("yhat", [BLOC, 1], dt.int32, kind="ExternalOutput")

    with tile.TileContext(nc) as tc:
        with (
            tc.tile_pool(name="consts", bufs=1) as cst,
            tc.tile_pool(name="xtr", bufs=3) as xtp,
            tc.tile_pool(name="mrow", bufs=2) as mrp,
            tc.tile_pool(name="acts", bufs=3) as actp,
            tc.tile_pool(name="small", bufs=3) as smp,
            tc.tile_pool(name="mpart", bufs=2) as mpp,
            tc.tile_pool(name="tail", bufs=2) as tlp,
            tc.tile_pool(name="psmm", bufs=2, space="PSUM") as psmm,
            tc.tile_pool(name="pss", bufs=2, space="PSUM") as pss,
            tc.tile_pool(name="pswr", bufs=2, space="PSUM") as pswr,
        ):
            # ---- load replicated constants -------------------------------
            w1t = cst.tile([128, 8, D], dt.bfloat16)
            nc.gpsimd.dma_start(w1t[:], w1.ap().rearrange("(c p) d -> p c d", p=128))
            w2t = cst.tile([128, 2, D], dt.bfloat16)
            nc.gpsimd.dma_start(w2t[:], w2.ap().rearrange("(c p) d -> p c d", p=128))
            w3t = cst.tile([128, 2], dt.bfloat16)
            nc.gpsimd.dma_start(w3t[:], w3p.ap())
            abt = cst.tile([128, 2, 2], dt.float32)
            nc.gpsimd.dma_start(abt[:], abp.ap())
            b2t = cst.tile([128, 2], dt.float32)
            nc.gpsimd.dma_start(b2t[:], b2p.ap())
            b3t = cst.tile([1, 1], dt.float32)
            nc.gpsimd.dma_start(b3t[:], b3p.ap())
            c1t = cst.tile([128, 8, D], dt.float32)
            nc.gpsimd.dma_start(c1t[:], c1.ap().rearrange("(c p) d -> p c d", p=128))
            c2t = cst.tile([128, 2, D], dt.float32)
            nc.gpsimd.dma_start(c2t[:], c2.ap().rearrange("(c p) d -> p c d", p=128))
            c3t = cst.tile([128, 2, 2], dt.float32)
            nc.gpsimd.dma_start(c3t[:], c3p.ap())
            c1rt = cst.tile([BLOC, D], dt.float32)
            nc.gpsimd.dma_start(c1rt[:], c1r.ap())
            c2rt = cst.tile([BLOC, D], dt.float32)
            nc.gpsimd.dma_start(c2rt[:], c2r.ap())
            c3rt = cst.tile([BLOC, 2], dt.float32)
            nc.gpsimd.dma_start(c3rt[:], c3r.ap())
            onet = cst.tile([1, 128], dt.bfloat16)
            nc.gpsimd.dma_start(onet[:], onesc.ap())
            oneft = cst.tile([1, 128], dt.float32)
            nc.gpsimd.dma_start(oneft[:], onescf.ap())
            id2t = cst.tile([2, 2], dt.float32)
            nc.gpsimd.dma_start(id2t[:], id2.ap())

            # mask rows for both slides up front (keeps the DMA stream
            # transpose-only afterwards)
            maskrows = []
            for s in range(BLOC):
                mr = mrp.tile([1, N], dt.bfloat16, tag="maskrow")
                nc.gpsimd.dma_start(mr[:], maskn[s])
                maskrows.append(mr)

            # normalized M^T columns for both slides: [128, chunk, slide]
            mtall = cst.tile([128, 8, BLOC], dt.float32)

            for s in range(BLOC):
                maskrow = maskrows[s]
                dent = smp.tile([1, NGRP], dt.float32, tag="dent")
                mpart = mpp.tile([128, 8, NGRP], dt.float32, tag="mpart")

                xthalf = {}
                for g in range(NGRP):
                    h_idx = (PG * g) // HPG
                    if (PG * g) % HPG == 0:
                        xth = xtp.tile([128, 8, HPG], dt.bfloat16, tag="xt")
                        for c in range(8):
                            nc.sync.dma_start_transpose(
                                xth[:, c, :],
                                xn[s, HPG * h_idx:HPG * (h_idx + 1),
                                   128 * c:128 * (c + 1)],
                            )
                        xthalf[h_idx] = xth
                    xtt = xthalf[h_idx]
                    goff = (PG * g) % HPG

                    # ---- layer 1: hT = x @ W1 (feature-major) -----------
                    hps = psmm.tile([128, 2, PG], dt.float32, tag="mm")
                    for dh in range(2):
                        for c in range(8):
                            nc.tensor.matmul(
                                hps[:, dh, :],
                                lhsT=w1t[:, c, 128 * dh:128 * (dh + 1)],
                                rhs=xtt[:, c, goff:goff + PG],
                                start=(c == 0), stop=(c == 7),
                            )
                    hs = actp.tile([128, 2, PG], dt.bfloat16, tag="hs")
                    for dh in range(2):
                        nc.scalar.activation(
                            hs[:, dh, :], hps[:, dh, :], AF.Prelu,
                            bias=abt[:, dh, 1:2], scale=abt[:, dh, 0:1], alpha=0.2,
                        )

                    # ---- layer 2 ----------------------------------------
                    gps = psmm.tile([128, 2, PG], dt.float32, tag="mm")
                    for eh in range(2):
                        for c in range(2):
                            nc.tensor.matmul(
                                gps[:, eh, :],
                                lhsT=w2t[:, c, 128 * eh:128 * (eh + 1)],
                                rhs=hs[:, c, :],
                                start=(c == 0), stop=(c == 1),
                            )
                    gs = actp.tile([128, 2, PG], dt.bfloat16, tag="gs")
                    for eh in range(2):
                        nc.scalar.activation(
                            gs[:, eh, :], gps[:, eh, :], AF.Prelu,
                            bias=b2t[:, eh:eh + 1], scale=1.0, alpha=0.2,
                        )

                    # ---- layer 3 + mask: scores row [1, PG] -------------
                    sps = pss.tile([1, PG], dt.float32, tag="srow")
                    for c in range(2):
                        nc.tensor.matmul(
                            sps[:],
                            lhsT=w3t[:, c:c + 1],
                            rhs=gs[:, c, :],
                            start=(c == 0), stop=False,
                        )
                    nc.tensor.matmul(
                        sps[:],
                        lhsT=onet[0:1, 0:1],
                        rhs=maskrow[0:1, PG * g:PG * (g + 1)],
                        start=False, stop=True,
                    )

                    # ---- w = exp(s + b3); denom partial -----------------
                    wrowb = smp.tile([1, PG], dt.bfloat16, tag="wrowb")
                    nc.scalar.activation(wrowb[:], sps[:], AF.Exp,
                                         bias=b3t[0:1, 0:1], scale=1.0,
                                         accum_out=dent[0:1, g:g + 1])

                    # ---- broadcast w across partitions (PE outer) -------
                    wrp = pswr.tile([128, PG], dt.float32, tag="wrep")
                    nc.tensor.matmul(wrp[:], lhsT=onet[0:1, :],
                                     rhs=wrowb[:], start=True, stop=True)

                    # ---- pooling on DVE: M^T partials via accum_out -----
                    for c in range(8):
                        junk = smp.tile([128, PG], dt.bfloat16, tag="junk")
                        nc.vector.scalar_tensor_tensor(
                            out=junk[:], in0=xtt[:, c, goff:goff + PG],
                            scalar=1.0, in1=wrp[:],
                            op0=OP.mult, op1=OP.mult,
                            accum_out=mpart[:, c, g:g + 1],
                        )

                # ---- slide tail: M^T = sum over groups, normalize -------
                den = tlp.tile([1, 1], dt.float32, tag="den")
                nc.vector.reduce_sum(den[:], dent[:], axis=AX.X)
                rden = tlp.tile([1, 1], dt.float32, tag="rden")
                nc.vector.reciprocal(rden[:], den[:])
                rvp = pss.tile([128, 1], dt.float32, tag="srow")
                nc.tensor.matmul(rvp[:], lhsT=oneft[0:1, :], rhs=rden[:],
                                 start=True, stop=True)
                rvs = tlp.tile([128, 1], dt.float32, tag="rvs")
                nc.vector.tensor_copy(rvs[:], rvp[:])
                msum = tlp.tile([128, 8, 1], dt.float32, tag="msum")
                nc.vector.reduce_sum(msum[:], mpart[:], axis=AX.X)
                nc.vector.tensor_scalar(
                    out=mtall[:, :, s:s + 1], in0=msum[:], scalar1=rvs[:],
                    scalar2=None, op0=OP.mult)

            # ---- classifier (both slides at once, fp32) -----------------
            z1p = psmm.tile([BLOC, D], dt.float32, tag="mm")
            for c in range(8):
                nc.tensor.matmul(z1p[:], lhsT=mtall[:, c, :], rhs=c1t[:, c, :],
                                 start=(c == 0), stop=(c == 7))
            z1a = tlp.tile([BLOC, D], dt.float32, tag="z1a")
            nc.vector.tensor_tensor(out=z1a[:], in0=z1p[:], in1=c1rt[:],
                                    op=OP.add)
            z1s = tlp.tile([BLOC, D], dt.float32, tag="z1s")
            nc.vector.tensor_scalar_max(z1s[:], z1a[:], 0.0)
            z1tp = psmm.tile([128, 2, BLOC], dt.float32, tag="mm")
            for c in range(2):
                nc.tensor.transpose(z1tp[:, c, :], z1s[:, 128 * c:128 * (c + 1)],
                                    id2t[:])
            z1tsb = tlp.tile([128, 2, BLOC], dt.float32, tag="z1t")
            nc.vector.tensor_copy(z1tsb[:], z1tp[:])

            z2p = pss.tile([BLOC, D], dt.float32, tag="srow")
            for c in range(2):
                nc.tensor.matmul(z2p[:], lhsT=z1tsb[:, c, :], rhs=c2t[:, c, :],
                                 start=(c == 0), stop=(c == 1))
            z2a = tlp.tile([BLOC, D], dt.float32, tag="z2a")
            nc.vector.tensor_tensor(out=z2a[:], in0=z2p[:], in1=c2rt[:],
                                    op=OP.add)
            z2s = tlp.tile([BLOC, D], dt.float32, tag="z2s")
            nc.vector.tensor_scalar_max(z2s[:], z2a[:], 0.0)
            z2tp = psmm.tile([128, 2, BLOC], dt.float32, tag="mm")
            for c in range(2):
                nc.tensor.transpose(z2tp[:, c, :], z2s[:, 128 * c:128 * (c + 1)],
                                    id2t[:])
            z2tsb = tlp.tile([128, 2, BLOC], dt.float32, tag="z2t")
            nc.vector.tensor_copy(z2tsb[:], z2tp[:])

            lgp = pss.tile([BLOC, 2], dt.float32, tag="srow")
            for c in range(2):
                nc.tensor.matmul(lgp[:], lhsT=z2tsb[:, c, :], rhs=c3t[:, c, :],
                                 start=(c == 0), stop=(c == 1))
            lg = tlp.tile([BLOC, 2], dt.float32, tag="lg")
            nc.vector.tensor_tensor(out=lg[:], in0=lgp[:], in1=c3rt[:], op=OP.add)

            diff = tlp.tile([BLOC, 1], dt.float32, tag="diff")
            nc.vector.tensor_tensor(out=diff[:], in0=lg[:, 1:2], in1=lg[:, 0:1],
                                    op=OP.subtract)
            ex = tlp.tile([BLOC, 1], dt.float32, tag="ex")
            nc.scalar.activation(ex[:], diff[:], AF.Exp, bias=0.0, scale=1.0)
            t1 = tlp.tile([BLOC, 1], dt.float32, tag="t1")
            nc.vector.tensor_scalar_add(t1[:], ex[:], 1.0)
            rr = tlp.tile([BLOC, 1], dt.float32, tag="rr")
            nc.vector.reciprocal(rr[:], t1[:])

            ypt = tlp.tile([BLOC, 2], dt.float32, tag="ypt")
            nc.vector.tensor_copy(ypt[:, 0:1], rr[:])          # p0 = 1/(1+e)
            nc.vector.tensor_tensor(out=ypt[:, 1:2], in0=ex[:], in1=rr[:],
                                    op=OP.mult)                # p1 = e/(1+e)
            yhf = tlp.tile([BLOC, 1], dt.float32, tag="yhf")
            nc.vector.tensor_scalar(out=yhf[:], in0=diff[:], scalar1=0.0,
                                    scalar2=None, op0=OP.is_gt)
            yht = tlp.tile([BLOC, 1], dt.int32, tag="yht")
            nc.vector.tensor_copy(yht[:], yhf[:])

            nc.sync.dma_start(yprob.ap(), ypt[:])
            nc.sync.dma_start(yhat.ap(), yht[:])

    nc.compile()
    return nc


def _prep(inputs):
    f32 = np.float32
    x = np.ascontiguousarray(np.asarray(inputs["x"], f32))
    mask = np.asarray(inputs["mask"])
    W1 = np.asarray(inputs["W1"], f32)
    b1 = np.asarray(inputs["b1"], f32)
    gam = np.asarray(inputs["bn_gamma"], f32)
    bet = np.asarray(inputs["bn_beta"], f32)
    mu = np.asarray(inputs["bn_mean"], f32)
    var = np.asarray(inputs["bn_var"], f32)
    W2 = np.asarray(inputs["W2"], f32)
    b2 = np.asarray(inputs["b2"], f32)
    W3 = np.asarray(inputs["W3"], f32)
    b3 = np.asarray(inputs["b3"], f32)
    C1 = np.asarray(inputs["C1"], f32)
    c1 = np.asarray(inputs["c1"], f32)
    C2 = np.asarray(inputs["C2"], f32)
    c2 = np.asarray(inputs["c2"], f32)
    C3 = np.asarray(inputs["C3"], f32)
    c3 = np.asarray(inputs["c3"], f32)

    A = gam / np.sqrt(var + EPS)                    # BN scale
    Bv = (b1 - mu) * A + bet                        # BN bias (b1 folded)

    xb = x.astype(_BF16)
    maskneg = np.where(mask > 0, 0.0, MASKNEG).astype(_BF16)
    shared = {
        "w1": W1.astype(_BF16),
        "w2": W2.astype(_BF16),
        "w3p": np.ascontiguousarray(W3.reshape(2, 128).T).astype(_BF16),
        "abp": np.ascontiguousarray(
            np.stack([A.reshape(2, 128).T, Bv.reshape(2, 128).T], axis=2)),
        "b2p": np.ascontiguousarray(b2.reshape(2, 128).T),
        "b3p": np.full((1, 1), b3[0], f32),
        "c1": C1,
        "c2": C2,
        "c3p": np.ascontiguousarray(C3.reshape(2, 128, 2).transpose(1, 0, 2)),
        "c1r": np.broadcast_to(c1, (BLOC, D)).copy(),
        "c2r": np.broadcast_to(c2, (BLOC, D)).copy(),
        "c3r": np.broadcast_to(c3, (BLOC, 2)).copy(),
        "onesc": np.ones((1, 128), _BF16),
        "onescf": np.ones((1, 128), f32),
        "id2": np.eye(2, dtype=f32),
    }
    in_maps = []
    for k in range(NCORES):
        sl = slice(BLOC * k, BLOC * (k + 1))
        m = dict(shared)
        m["xn"] = np.ascontiguousarray(xb[sl])
        m["maskn"] = np.ascontiguousarray(maskneg[sl].reshape(BLOC, 1, N))
        in_maps.append(m)
    return in_maps


def kernel(**inputs):
    from concourse.bass_utils import run_bass_kernel_spmd

    if "nc" not in _CACHE:
        _CACHE["nc"] = _build_nc()
    nc = _CACHE["nc"]

    in_maps = _prep(inputs)
    res = run_bass_kernel_spmd(nc, in_maps, core_ids=list(range(NCORES)))
    yprob = np.concatenate([r["yprob"] for r in res.results], axis=0)
    yhat = np.concatenate([r["yhat"][:, 0] for r in res.results], axis=0)
    return yprob.astype(np.float32), yhat.astype(np.int32)
